# revision 39
# baseline (speedup 1.0000x reference)
"""Causal self-attention on 8 TRN2 NeuronCores.

Problem (hardcoded): B=4, T=2048, C=1024, H=16 heads, D=64.
  qkv = x @ W_in + b_in ; causal softmax attention ; out = y @ W_out + b_out

Sharding: core c handles batch b = c//2 and head-group g = c%2 (8 heads).
Each core computes its partial out-projection (sum over its heads' columns);
the host adds the two partials per batch plus b_out (+ the v-bias term,
which commutes through the softmax average). No device collectives.

Device design (fp8 projections, fp16 attention core):
  - QKV projection runs as fp8e4 DoubleRow matmuls: x and W_in are host-split
    into hi+lo e4m3 parts (pre-scaled by 2^5 / 2^7 so values sit mid-range,
    away from fp8 subnormals), and 3 of the 4 cross terms are accumulated
    (hi*hi + lo*hi + hi*lo). DoubleRow contracts two 128-chunks per
    instruction at 0.5 cycles/row -> 1.33x the fp16 rate at ~0.1% error.
    The 2^-12 unscale folds into the bias-add (tensor_scalar mult+add).
  - Attention core is fp16: q/k (1/sqrt(D) folded into W_q), P = exp(S) from
    ACT directly in fp16 (max P ~500 fits fp16; no max-subtraction), v
    carries a ones-column so the PV matmul emits softmax denominators free.
  - Scores computed transposed: S^T[k, q] = k . q, so exp(S^T) = P^T feeds
    the PV matmul directly -- no on-chip transposes anywhere.
  - Causal masking multiplies only the 128-wide diagonal block of each
    diagonal chunk (columns left of it are skipped via col0, columns right
    of it are fully valid).
  - Causal pipeline: projection of query-window w+1 and the out-projection
    of window w-1 are emitted as PE "filler" units interleaved into the
    ACT(exp)-bound attention stream of window w, keeping PE dense.
"""

import sys

for _p in ("/opt/trn_rl_repo", "/root/.axon_site/_ro/trn_rl_repo"):
    if _p not in sys.path:
        sys.path.append(_p)

import numpy as np

B, T, C = 4, 2048, 1024
H = 16  # total heads
HL = 8  # heads per core
D = 64  # head dim
P = 128
KO = C // P  # 8 contraction chunks
KOP = KO // 2  # 4 DoubleRow chunk-pairs
TQ = 512  # query-window width
NTQ = T // TQ  # 4 windows
FQK = 2 * HL * D  # 1024 (q block then k block)
FV = HL * D  # 512

# fp8 pre-scales (powers of two; exact)
SX = 32.0  # x scale
SW = 128.0  # weight scale
UNSCALE = 1.0 / (SX * SW)  # folded into the bias-add after projection

_CACHE = {}


def _build():
    import concourse.mybir as mybir
    import concourse.tile as tile
    from concourse import bacc

    f32 = mybir.dt.float32
    f16 = mybir.dt.float16
    f8 = mybir.dt.float8e4
    DR = mybir.MatmulPerfMode.DoubleRow
    Exp = mybir.ActivationFunctionType.Exp
    mult = mybir.AluOpType.mult
    add = mybir.AluOpType.add
    subtract = mybir.AluOpType.subtract

    nc = bacc.Bacc("TRN2", target_bir_lowering=False, debug=False, num_devices=8)

    xh_d = nc.dram_tensor("xh", [C, T], f8, kind="ExternalInput")
    xl_d = nc.dram_tensor("xl", [C, T], f8, kind="ExternalInput")
    wqkh_d = nc.dram_tensor("wqkh", [C, FQK], f8, kind="ExternalInput")
    wqkl_d = nc.dram_tensor("wqkl", [C, FQK], f8, kind="ExternalInput")
    wvh_d = nc.dram_tensor("wvh", [C, FV], f8, kind="ExternalInput")
    wvl_d = nc.dram_tensor("wvl", [C, FV], f8, kind="ExternalInput")
    woh_d = nc.dram_tensor("woh", [HL * D, C], f8, kind="ExternalInput")
    wol_d = nc.dram_tensor("wol", [HL * D, C], f8, kind="ExternalInput")
    b_qk_d = nc.dram_tensor("b_qk", [FQK], f32, kind="ExternalInput")
    masks_d = nc.dram_tensor("masks", [P, 896], f16, kind="ExternalInput")
    out_d = nc.dram_tensor("out", [T, C], f32, kind="ExternalOutput")

    with tile.TileContext(nc) as tc:
        import contextlib
        from collections import deque

        ctx = contextlib.ExitStack()
        with ctx:
            persist = ctx.enter_context(tc.tile_pool(name="persist", bufs=1))
            qT_pool = ctx.enter_context(tc.tile_pool(name="qT", bufs=2))
            xT_pool = ctx.enter_context(tc.tile_pool(name="xT", bufs=2))
            pT_pool = ctx.enter_context(tc.tile_pool(name="pT", bufs=8))
            sm = ctx.enter_context(tc.tile_pool(name="sm", bufs=4))
            yT_pool = ctx.enter_context(tc.tile_pool(name="yT", bufs=4))
            o_pool = ctx.enter_context(tc.tile_pool(name="o", bufs=6))

            # ---- weights + first x window, in first-use order ----
            wqkh_t = persist.tile([P, KO, FQK], f8)
            wqkl_t = persist.tile([P, KO, FQK], f8)
            xh0, xl0 = [], []
            for m in range(KOP):
                nc.sync.dma_start(
                    wqkh_t[:, 2 * m : 2 * m + 2],
                    wqkh_d[256 * m : 256 * (m + 1)].rearrange(
                        "(two p) f -> p two f", p=P
                    ),
                )
                t_ = xT_pool.tile([P, 2, TQ], f8, tag=f"xh{m}", name=f"xh0_{m}")
                nc.gpsimd.dma_start(
                    t_,
                    xh_d[256 * m : 256 * (m + 1), 0:TQ].rearrange(
                        "(two p) t -> p two t", p=P
                    ),
                )
                xh0.append(t_)
            for m in range(KOP):
                nc.sync.dma_start(
                    wqkl_t[:, 2 * m : 2 * m + 2],
                    wqkl_d[256 * m : 256 * (m + 1)].rearrange(
                        "(two p) f -> p two f", p=P
                    ),
                )
                t_ = xT_pool.tile([P, 2, TQ], f8, tag=f"xl{m}", name=f"xl0_{m}")
                # split the lo-x loads across both queues so the last chunk
                # lands early (the w0 projection is paced by it)
                q_ = nc.gpsimd if m < 2 else nc.sync
                q_.dma_start(
                    t_,
                    xl_d[256 * m : 256 * (m + 1), 0:TQ].rearrange(
                        "(two p) t -> p two t", p=P
                    ),
                )
                xl0.append(t_)
            b_qk_sb = persist.tile([P, KO], f32)
            nc.sync.dma_start(b_qk_sb, b_qk_d.rearrange("(fo p) -> p fo", p=P))
            # preload the Exp activation table off the critical path
            warm = persist.tile([1, 1], f32)
            nc.scalar.activation(warm, b_qk_sb[0:1, 0:1], Exp)
            wvh_t = persist.tile([P, KO, FV], f8)
            wvl_t = persist.tile([P, KO, FV], f8)
            for m in range(KOP):
                nc.gpsimd.dma_start(
                    wvh_t[:, 2 * m : 2 * m + 2],
                    wvh_d[256 * m : 256 * (m + 1)].rearrange(
                        "(two p) f -> p two f", p=P
                    ),
                )
            for m in range(KOP):
                nc.gpsimd.dma_start(
                    wvl_t[:, 2 * m : 2 * m + 2],
                    wvl_d[256 * m : 256 * (m + 1)].rearrange(
                        "(two p) f -> p two f", p=P
                    ),
                )
            mask_sb = persist.tile([P, 896], f16)
            nc.sync.dma_start(mask_sb, masks_d[:])

            # per-window persistent activations
            kT_w = []  # [p(d-pair), kfo(4), TQ] per window, fp16
            v65_w = []  # [p(key), t4(4), HL, 65] per window, fp16 (v | 1)
            for w in range(NTQ):
                kT_w.append(
                    persist.tile([P, 4, TQ], f16, tag=f"kT{w}", name=f"kT{w}")
                )
                v65_w.append(
                    persist.tile(
                        [P, 4, HL, D + 1], f16, tag=f"v65{w}", name=f"v65{w}"
                    )
                )
                nc.vector.memset(v65_w[w][:, :, :, D], 1.0 / 16.0)
            woh_t = persist.tile([P, 4, C], f8)  # [p, do(pair j), n]
            wol_t = persist.tile([P, 4, C], f8)
            for do in range(4):
                nc.gpsimd.dma_start(woh_t[:, do], woh_d[do * P : (do + 1) * P, :])
            for do in range(4):
                nc.gpsimd.dma_start(wol_t[:, do], wol_d[do * P : (do + 1) * P, :])

            # ---------------- unit builders ----------------
            def load_x(w):
                xh_ts, xl_ts = [], []
                for m in range(KOP):
                    t_ = xT_pool.tile([P, 2, TQ], f8, tag=f"xh{m}")
                    nc.sync.dma_start(
                        t_,
                        xh_d[
                            256 * m : 256 * (m + 1), w * TQ : (w + 1) * TQ
                        ].rearrange("(two p) t -> p two t", p=P),
                    )
                    xh_ts.append(t_)
                for m in range(KOP):
                    t_ = xT_pool.tile([P, 2, TQ], f8, tag=f"xl{m}")
                    nc.sync.dma_start(
                        t_,
                        xl_d[
                            256 * m : 256 * (m + 1), w * TQ : (w + 1) * TQ
                        ].rearrange("(two p) t -> p two t", p=P),
                    )
                    xl_ts.append(t_)
                return xh_ts, xl_ts

            def proj_qk_unit(w, fo, qT_next, xh_ts, xl_ts):
                def emit():
                    ps = ps_pj.tile([P, TQ], f32, tag="pj")
                    terms = [(wqkh_t, xh_ts), (wqkl_t, xh_ts), (wqkh_t, xl_ts)]
                    n = 0
                    for wt, xt in terms:
                        for m in range(KOP):
                            nc.tensor.matmul(
                                ps,
                                wt[:, 2 * m : 2 * m + 2, fo * P : (fo + 1) * P],
                                xt[m],
                                start=(n == 0),
                                stop=(n == 3 * KOP - 1),
                                perf_mode=DR,
                            )
                            n += 1
                    dst = qT_next[:, fo] if fo < 4 else kT_w[w][:, fo - 4]
                    nc.vector.tensor_scalar(
                        dst, ps, UNSCALE, b_qk_sb[:, fo : fo + 1], mult, add
                    )

                return emit

            def proj_v_unit(w, t4, xh_ts, xl_ts):
                def emit():
                    ps = ps_pj.tile([P, TQ], f32, tag="pj")
                    terms = [(xh_ts, wvh_t), (xl_ts, wvh_t), (xh_ts, wvl_t)]
                    n = 0
                    for xt, wt in terms:
                        for m in range(KOP):
                            nc.tensor.matmul(
                                ps,
                                xt[m][:, :, t4 * P : (t4 + 1) * P],
                                wt[:, 2 * m : 2 * m + 2, :],
                                start=(n == 0),
                                stop=(n == 3 * KOP - 1),
                                perf_mode=DR,
                            )
                            n += 1
                    nc.vector.tensor_scalar(
                        v65_w[w][:, t4, :, :D],
                        ps.rearrange("p (h d) -> p h d", h=HL),
                        UNSCALE,
                        None,
                        mult,
                    )

                return emit

            def op_unit(tq, ts_, yh_win, yl_win, scalar_copy=False):
                def emit():
                    t0 = tq * TQ + ts_ * P
                    for n in range(2):
                        ps = ps_pj.tile([P, TQ], f32, tag="pj")
                        terms = [(yh_win, woh_t), (yl_win, woh_t), (yh_win, wol_t)]
                        k = 0
                        for m in range(2):
                            for yt, wt in terms:
                                nc.tensor.matmul(
                                    ps,
                                    yt[:, 2 * m : 2 * m + 2, ts_ * P : (ts_ + 1) * P],
                                    wt[:, 2 * m : 2 * m + 2, n * 512 : (n + 1) * 512],
                                    start=(k == 0),
                                    stop=(k == 5),
                                    perf_mode=DR,
                                )
                                k += 1
                        o_sb = o_pool.tile([P, 512], f32, tag="o")
                        if scalar_copy and n == 0:
                            nc.scalar.copy(o_sb, ps)
                        else:
                            nc.vector.tensor_copy(o_sb, ps)
                        q_ = nc.gpsimd if (scalar_copy and n == 1) else nc.sync
                        q_.dma_start(
                            out_d[t0 : t0 + P, n * 512 : (n + 1) * 512], o_sb
                        )

                return emit

            # paced filler drain
            class Pacer:
                def __init__(self, fillers, total_slots, backload=1.0, reserve=0):
                    self.fillers = deque(fillers)
                    self.total = max(1, total_slots)
                    self.n = len(fillers)
                    self.slot = 0
                    self.done = 0
                    self.backload = backload
                    self.reserve = reserve

                def tick(self):
                    self.slot += 1
                    want = min(
                        int(self.n * (self.slot / self.total) ** self.backload),
                        self.n - self.reserve,
                    )
                    while self.done < want and self.fillers:
                        self.fillers.popleft()()
                        self.done += 1

                def drain(self):
                    while self.fillers:
                        self.fillers.popleft()()

            def att_pair(tq, j, qT_cur, yh_win, yl_win, pacer, last=False):
                """Heads 2j (partitions 0:64) and 2j+1 (64:128): S in fp16,
                exp on ACT straight to fp16 pT, PV with moving = pT."""
                nchunks = 4 * (tq + 1)
                hA, hB = 2 * j, 2 * j + 1
                qA = qT_cur[0:D, j, :]
                qB = qT_cur[D:P, j, :]
                psyA = ps_y.tile([D + 1, TQ], f32, tag="psyA", name="psyA")
                psyB = ps_y.tile([D + 1, TQ], f32, tag="psyB", name="psyB")
                from collections import deque as _dq

                pvq = _dq()
                for i in range(nchunks):
                    i4 = i - 4 * tq
                    diag = 0 <= i4 < 4
                    col0 = P * i4 if diag else 0
                    pss = ps_s.tile([P, 2, TQ], f32, tag="ps_s")
                    kslice = slice((i % 4) * P, (i % 4 + 1) * P)
                    nc.tensor.matmul(
                        pss[:, 0, col0:TQ],
                        kT_w[i // 4][0:D, j, kslice],
                        qA[:, col0:TQ],
                        start=True,
                        stop=True,
                    )
                    nc.tensor.matmul(
                        pss[:, 1, col0:TQ],
                        kT_w[i // 4][D:P, j, kslice],
                        qB[:, col0:TQ],
                        start=True,
                        stop=True,
                    )
                    pT = pT_pool.tile([P, 2, TQ], f16, tag="pT")
                    nc.scalar.activation(
                        pT[:, :, col0:TQ], pss[:, :, col0:TQ], Exp
                    )
                    if diag:
                        # only the diagonal 128-col block needs masking
                        nc.vector.tensor_tensor(
                            pT[:, :, col0 : col0 + P],
                            pT[:, :, col0 : col0 + P],
                            mask_sb[:, 384 : 384 + P]
                            .unsqueeze(1)
                            .to_broadcast((P, 2, P)),
                            mult,
                        )
                    pvq.append((pT, i, col0))
                    if len(pvq) > 6:
                        _pv(pvq.popleft(), psyA, psyB, nchunks, hA, hB)
                    pacer.tick()
                while pvq:
                    _pv(pvq.popleft(), psyA, psyB, nchunks, hA, hB)
                # normalize: copy psy to SBUF first (frees the PSUM bank for
                # the next pair), then reciprocal + GpSimd partition
                # broadcast + multiply, A/B interleaved to hide latency
                stage = []
                for psy, pb in ((psyA, 0), (psyB, D)):
                    if last:
                        # final pair: nobody needs the PSUM bank next, so
                        # normalize straight out of PSUM (shorter tail chain)
                        psy_sb = psy
                    else:
                        psy_sb = sm.tile(
                            [D + 1, TQ], f16, tag="psy_sb", name="psy_sb"
                        )
                        nc.vector.tensor_copy(psy_sb, psy)
                    rec_bc = sm.tile([D, TQ], f32, tag="rec_bc")
                    stage.append((psy_sb, rec_bc, pb))
                for psy_sb, rec_bc, pb in stage:
                    with nc.allow_low_precision(reason="softmax denominators"):
                        nc.vector.reciprocal(rec_bc[0:1, :], psy_sb[D : D + 1, :])
                for psy_sb, rec_bc, pb in stage:
                    nc.gpsimd.partition_broadcast(rec_bc, rec_bc[0:1, :], channels=D)
                y16 = sm.tile([P, TQ], f16, tag="y16", name="y16")
                if last:
                    # col-sliced so the tail out-projections start per q-tile
                    for ts_ in range(4):
                        cs = slice(ts_ * P, (ts_ + 1) * P)
                        for psy_sb, rec_bc, pb in stage:
                            nc.vector.tensor_tensor(
                                y16[pb : pb + D, cs],
                                psy_sb[:D, cs],
                                rec_bc[:, cs],
                                mult,
                            )
                            nc.vector.tensor_copy(
                                yh_win[pb : pb + D, j, cs], y16[pb : pb + D, cs]
                            )
                            nc.vector.tensor_tensor(
                                yl_win[pb : pb + D, j, cs],
                                y16[pb : pb + D, cs],
                                yh_win[pb : pb + D, j, cs],
                                subtract,
                            )
                else:
                    for psy_sb, rec_bc, pb in stage:
                        nc.vector.tensor_tensor(
                            y16[pb : pb + D, :], psy_sb[:D, :], rec_bc, mult
                        )
                        # split into fp8 hi+lo (16*y fits e4m3 range);
                        # alternate engines so DVE and Pool share the work
                        eng = nc.vector if pb == 0 else nc.gpsimd
                        eng.tensor_copy(
                            yh_win[pb : pb + D, j, :], y16[pb : pb + D, :]
                        )
                        eng.tensor_tensor(
                            yl_win[pb : pb + D, j, :],
                            y16[pb : pb + D, :],
                            yh_win[pb : pb + D, j, :],
                            subtract,
                        )

            def _pv(prev, psyA, psyB, nchunks, hA, hB):
                pT, i, col0 = prev
                nc.tensor.matmul(
                    psyA[:, col0:TQ],
                    v65_w[i // 4][:, i % 4, hA],
                    pT[:, 0, col0:TQ],
                    start=(i == 0),
                    stop=(i == nchunks - 1),
                )
                nc.tensor.matmul(
                    psyB[:, col0:TQ],
                    v65_w[i // 4][:, i % 4, hB],
                    pT[:, 1, col0:TQ],
                    start=(i == 0),
                    stop=(i == nchunks - 1),
                )

            # ---------------- emission ----------------
            # window-0 projection: ko-pair-outer so PE starts on first chunks
            qT_cur = qT_pool.tile([P, 4, TQ], tag="qT", dtype=f16)
            with tc.tile_pool(name="pj0", bufs=1, space="PSUM") as pj0:
                ps_fo = [
                    pj0.tile([P, TQ], f32, tag=f"pj0_{fo}", name=f"pj0_{fo}")
                    for fo in range(KO)
                ]
                sweeps = [(wqkh_t, xh0), (wqkl_t, xh0), (wqkh_t, xl0)]
                fo_last = [4, 0, 1, 2, 3, 5, 6, 7]  # kT chunk 0 + qT first
                for s, (wt, xt) in enumerate(sweeps):
                    for m in range(KOP):
                        last = s == 2 and m == KOP - 1
                        for fo in fo_last if last else range(KO):
                            nc.tensor.matmul(
                                ps_fo[fo],
                                wt[:, 2 * m : 2 * m + 2, fo * P : (fo + 1) * P],
                                xt[m],
                                start=(s == 0 and m == 0),
                                stop=last,
                                perf_mode=DR,
                            )
                            if last:
                                dst = (
                                    qT_cur[:, fo] if fo < 4 else kT_w[0][:, fo - 4]
                                )
                                nc.vector.tensor_scalar(
                                    dst,
                                    ps_fo[fo],
                                    UNSCALE,
                                    b_qk_sb[:, fo : fo + 1],
                                    mult,
                                    add,
                                )
                for t4 in range(4):
                    psv = pj0.tile([P, TQ], f32, tag=f"pj0_{t4}", name=f"pj0v_{t4}")
                    vsweeps = [(xh0, wvh_t), (xl0, wvh_t), (xh0, wvl_t)]
                    n = 0
                    for xt, wt in vsweeps:
                        for m in range(KOP):
                            nc.tensor.matmul(
                                psv,
                                xt[m][:, :, t4 * P : (t4 + 1) * P],
                                wt[:, 2 * m : 2 * m + 2, :],
                                start=(n == 0),
                                stop=(n == 3 * KOP - 1),
                                perf_mode=DR,
                            )
                            n += 1
                    nc.vector.tensor_scalar(
                        v65_w[0][:, t4, :, :D],
                        psv.rearrange("p (h d) -> p h d", h=HL),
                        UNSCALE,
                        None,
                        mult,
                    )
            ps_pj = ctx.enter_context(tc.tile_pool(name="ps_pj", bufs=2, space="PSUM"))
            ps_s = ctx.enter_context(tc.tile_pool(name="ps_s", bufs=2, space="PSUM"))
            ps_y = ctx.enter_context(tc.tile_pool(name="ps_y", bufs=1, space="PSUM"))

            yT_all = []
            for tq in range(NTQ):
                fillers = []
                qT_next = None
                if tq + 1 < NTQ:
                    xh_ts, xl_ts = load_x(tq + 1)
                    qT_next = qT_pool.tile([P, 4, TQ], tag="qT", dtype=f16)
                    # consumption order at window start: kT chunk, qT pair, v
                    for fo, t4 in ((4, 0), (0, 1), (5, 2), (1, 3)):
                        fillers.append(
                            proj_qk_unit(tq + 1, fo, qT_next, xh_ts, xl_ts)
                        )
                        fillers.append(proj_v_unit(tq + 1, t4, xh_ts, xl_ts))
                    for fo in (6, 2, 7, 3):
                        fillers.append(
                            proj_qk_unit(tq + 1, fo, qT_next, xh_ts, xl_ts)
                        )
                else:
                    # window 3 is ACT(exp)-bound: all deferred out-projections
                    # (windows 0-2) become its PE filler mass
                    for w in range(NTQ - 1):
                        for ts_ in range(4):
                            fillers.append(op_unit(w, ts_, *yT_all[w]))
                yh_win = yT_pool.tile([P, 4, TQ], f8, tag="yh", name="yh_win")
                yl_win = yT_pool.tile([P, 4, TQ], f8, tag="yl", name="yl_win")
                yT_all.append((yh_win, yl_win))
                pacer = Pacer(
                    fillers,
                    total_slots=4 * 4 * (tq + 1),
                    backload=1.0,
                    reserve=2 if tq == NTQ - 1 else 0,
                )
                if tq == 0 and fillers:
                    for _ in range(2):
                        pacer.fillers.popleft()()
                        pacer.done += 1
                for j in range(HL // 2):
                    att_pair(
                        tq,
                        j,
                        qT_cur,
                        yh_win,
                        yl_win,
                        pacer,
                        last=(tq == NTQ - 1 and j == HL // 2 - 1),
                    )
                pacer.drain()
                qT_cur = qT_next
            for ts_ in range(4):
                op_unit(NTQ - 1, ts_, *yT_all[NTQ - 1], scalar_copy=True)()

    nc.compile()
    return nc


def _get_nc():
    if "nc" not in _CACHE:
        _CACHE["nc"] = _build()
    return _CACHE["nc"]


def _split8(a, s):
    import ml_dtypes

    E4 = ml_dtypes.float8_e4m3
    a = np.asarray(a, np.float32) * np.float32(s)
    h = a.astype(E4)
    l = (a - h.astype(np.float32)).astype(E4)
    return np.ascontiguousarray(h), np.ascontiguousarray(l)


def kernel(x, W_in, b_in, W_out, b_out):
    from concourse.bass_utils import run_bass_kernel_spmd

    x = np.asarray(x, dtype=np.float32)
    W_in = np.asarray(W_in, dtype=np.float32)
    b_in = np.asarray(b_in, dtype=np.float32)
    W_out = np.asarray(W_out, dtype=np.float32)
    b_out = np.asarray(b_out, dtype=np.float32)

    scale = 1.0 / np.sqrt(D)

    # causal mask master: M[p, u] = 1 if u >= p + 384
    u = np.arange(896)[None, :]
    p = np.arange(P)[:, None]
    mask = (u >= p + 384).astype(np.float16)

    in_maps = []
    for c in range(8):
        b, g = c // 2, c % 2
        qc = slice(g * HL * D, (g + 1) * HL * D)
        kc = slice(C + g * HL * D, C + (g + 1) * HL * D)
        vc = slice(2 * C + g * HL * D, 2 * C + (g + 1) * HL * D)
        w_qk = np.concatenate([W_in[:, qc] * scale, W_in[:, kc]], axis=1)
        b_qk = np.concatenate([b_in[qc] * scale, b_in[kc]]).astype(np.float32)
        wqkh, wqkl = _split8(w_qk, SW)
        wvh, wvl = _split8(W_in[:, vc], SW)
        woh, wol = _split8(W_out[g * HL * D : (g + 1) * HL * D, :], SW)
        xh, xl = _split8(x[b].T, SX)
        in_maps.append(
            {
                "xh": xh,
                "xl": xl,
                "wqkh": wqkh,
                "wqkl": wqkl,
                "wvh": wvh,
                "wvl": wvl,
                "woh": woh,
                "wol": wol,
                "b_qk": b_qk,
                "masks": mask,
            }
        )

    nc = _get_nc()
    res = run_bass_kernel_spmd(nc, in_maps, list(range(8)))

    # host epilogue: add halves, biases, and the v-bias term
    # (y = softmax(..)@(v + b_v) = y0 + b_v, so b_v @ W_out adds to out)
    bv_term = np.zeros(C, np.float64)
    for g in range(2):
        vc = slice(2 * C + g * HL * D, 2 * C + (g + 1) * HL * D)
        gsl = slice(g * HL * D, (g + 1) * HL * D)
        bv_term += b_in[vc].astype(np.float64) @ W_out[gsl].astype(np.float64)
    epilogue = (bv_term + b_out).astype(np.float32)

    out = np.empty((B, T, C), np.float32)
    unscale_out = np.float32(1.0 / (16.0 * SW))
    for b in range(B):
        out[b] = (
            res.results[2 * b]["out"] + res.results[2 * b + 1]["out"]
        ) * unscale_out + epilogue
    return out


if __name__ == "__main__":
    rng = np.random.default_rng(0)
    x = rng.standard_normal((B, T, C), dtype=np.float32)
    W_in = rng.standard_normal((C, 3 * C), dtype=np.float32) / np.sqrt(C)
    b_in = np.zeros(3 * C, np.float32)
    W_out = rng.standard_normal((C, C), dtype=np.float32) / np.sqrt(C)
    b_out = np.zeros(C, np.float32)
    y = kernel(x=x, W_in=W_in, b_in=b_in, W_out=W_out, b_out=b_out)
    print("ok", y.shape, y.dtype)


# revision 40
# speedup vs baseline: 1.0014x; 1.0014x over previous
"""Causal self-attention on 8 TRN2 NeuronCores.

Problem (hardcoded): B=4, T=2048, C=1024, H=16 heads, D=64.
  qkv = x @ W_in + b_in ; causal softmax attention ; out = y @ W_out + b_out

Sharding: core c handles batch b = c//2 and head-group g = c%2 (8 heads).
Each core computes its partial out-projection (sum over its heads' columns);
the host adds the two partials per batch plus b_out (+ the v-bias term,
which commutes through the softmax average). No device collectives.

Device design (fp8 projections, fp16 attention core):
  - QKV projection runs as fp8e4 DoubleRow matmuls: x and W_in are host-split
    into hi+lo e4m3 parts (pre-scaled by 2^5 / 2^7 so values sit mid-range,
    away from fp8 subnormals), and 3 of the 4 cross terms are accumulated
    (hi*hi + lo*hi + hi*lo). DoubleRow contracts two 128-chunks per
    instruction at 0.5 cycles/row -> 1.33x the fp16 rate at ~0.1% error.
    The 2^-12 unscale folds into the bias-add (tensor_scalar mult+add).
  - Attention core is fp16: q/k (1/sqrt(D) folded into W_q), P = exp(S) from
    ACT directly in fp16 (max P ~500 fits fp16; no max-subtraction), v
    carries a ones-column so the PV matmul emits softmax denominators free.
  - Scores computed transposed: S^T[k, q] = k . q, so exp(S^T) = P^T feeds
    the PV matmul directly -- no on-chip transposes anywhere.
  - Causal masking multiplies only the 128-wide diagonal block of each
    diagonal chunk (columns left of it are skipped via col0, columns right
    of it are fully valid).
  - Causal pipeline: projection of query-window w+1 and the out-projection
    of window w-1 are emitted as PE "filler" units interleaved into the
    ACT(exp)-bound attention stream of window w, keeping PE dense.
"""

import sys

for _p in ("/opt/trn_rl_repo", "/root/.axon_site/_ro/trn_rl_repo"):
    if _p not in sys.path:
        sys.path.append(_p)

import numpy as np

B, T, C = 4, 2048, 1024
H = 16  # total heads
HL = 8  # heads per core
D = 64  # head dim
P = 128
KO = C // P  # 8 contraction chunks
KOP = KO // 2  # 4 DoubleRow chunk-pairs
TQ = 512  # query-window width
NTQ = T // TQ  # 4 windows
FQK = 2 * HL * D  # 1024 (q block then k block)
FV = HL * D  # 512

# fp8 pre-scales (powers of two; exact)
SX = 32.0  # x scale
SW = 128.0  # weight scale
UNSCALE = 1.0 / (SX * SW)  # folded into the bias-add after projection

_CACHE = {}


def _build():
    import concourse.mybir as mybir
    import concourse.tile as tile
    from concourse import bacc

    f32 = mybir.dt.float32
    f16 = mybir.dt.float16
    f8 = mybir.dt.float8e4
    DR = mybir.MatmulPerfMode.DoubleRow
    Exp = mybir.ActivationFunctionType.Exp
    mult = mybir.AluOpType.mult
    add = mybir.AluOpType.add
    subtract = mybir.AluOpType.subtract

    nc = bacc.Bacc("TRN2", target_bir_lowering=False, debug=False, num_devices=8)

    xh_d = nc.dram_tensor("xh", [C, T], f8, kind="ExternalInput")
    xl_d = nc.dram_tensor("xl", [C, T], f8, kind="ExternalInput")
    wqkh_d = nc.dram_tensor("wqkh", [C, FQK], f8, kind="ExternalInput")
    wqkl_d = nc.dram_tensor("wqkl", [C, FQK], f8, kind="ExternalInput")
    wvh_d = nc.dram_tensor("wvh", [C, FV], f8, kind="ExternalInput")
    wvl_d = nc.dram_tensor("wvl", [C, FV], f8, kind="ExternalInput")
    woh_d = nc.dram_tensor("woh", [HL * D, C], f8, kind="ExternalInput")
    wol_d = nc.dram_tensor("wol", [HL * D, C], f8, kind="ExternalInput")
    b_qk_d = nc.dram_tensor("b_qk", [FQK], f32, kind="ExternalInput")
    masks_d = nc.dram_tensor("masks", [P, 896], f16, kind="ExternalInput")
    out_d = nc.dram_tensor("out", [T, C], f32, kind="ExternalOutput")

    with tile.TileContext(nc) as tc:
        import contextlib
        from collections import deque

        ctx = contextlib.ExitStack()
        with ctx:
            persist = ctx.enter_context(tc.tile_pool(name="persist", bufs=1))
            qT_pool = ctx.enter_context(tc.tile_pool(name="qT", bufs=2))
            xT_pool = ctx.enter_context(tc.tile_pool(name="xT", bufs=2))
            pT_pool = ctx.enter_context(tc.tile_pool(name="pT", bufs=8))
            sm = ctx.enter_context(tc.tile_pool(name="sm", bufs=4))
            yT_pool = ctx.enter_context(tc.tile_pool(name="yT", bufs=4))
            o_pool = ctx.enter_context(tc.tile_pool(name="o", bufs=6))

            # ---- weights + first x window, in first-use order ----
            wqkh_t = persist.tile([P, KO, FQK], f8)
            wqkl_t = persist.tile([P, KO, FQK], f8)
            xh0, xl0 = [], []
            for m in range(KOP):
                nc.sync.dma_start(
                    wqkh_t[:, 2 * m : 2 * m + 2],
                    wqkh_d[256 * m : 256 * (m + 1)].rearrange(
                        "(two p) f -> p two f", p=P
                    ),
                )
                t_ = xT_pool.tile([P, 2, TQ], f8, tag=f"xh{m}", name=f"xh0_{m}")
                nc.gpsimd.dma_start(
                    t_,
                    xh_d[256 * m : 256 * (m + 1), 0:TQ].rearrange(
                        "(two p) t -> p two t", p=P
                    ),
                )
                xh0.append(t_)
            for m in range(KOP):
                nc.sync.dma_start(
                    wqkl_t[:, 2 * m : 2 * m + 2],
                    wqkl_d[256 * m : 256 * (m + 1)].rearrange(
                        "(two p) f -> p two f", p=P
                    ),
                )
                t_ = xT_pool.tile([P, 2, TQ], f8, tag=f"xl{m}", name=f"xl0_{m}")
                # split the lo-x loads across both queues so the last chunk
                # lands early (the w0 projection is paced by it)
                q_ = nc.gpsimd if m < 2 else nc.sync
                q_.dma_start(
                    t_,
                    xl_d[256 * m : 256 * (m + 1), 0:TQ].rearrange(
                        "(two p) t -> p two t", p=P
                    ),
                )
                xl0.append(t_)
            b_qk_sb = persist.tile([P, KO], f32)
            nc.sync.dma_start(b_qk_sb, b_qk_d.rearrange("(fo p) -> p fo", p=P))
            # preload the Exp activation table off the critical path
            warm = persist.tile([1, 1], f32)
            nc.scalar.activation(warm, b_qk_sb[0:1, 0:1], Exp)
            wvh_t = persist.tile([P, KO, FV], f8)
            wvl_t = persist.tile([P, KO, FV], f8)
            for m in range(KOP):
                nc.gpsimd.dma_start(
                    wvh_t[:, 2 * m : 2 * m + 2],
                    wvh_d[256 * m : 256 * (m + 1)].rearrange(
                        "(two p) f -> p two f", p=P
                    ),
                )
            for m in range(KOP):
                nc.gpsimd.dma_start(
                    wvl_t[:, 2 * m : 2 * m + 2],
                    wvl_d[256 * m : 256 * (m + 1)].rearrange(
                        "(two p) f -> p two f", p=P
                    ),
                )
            mask_sb = persist.tile([P, 896], f16)
            nc.sync.dma_start(mask_sb, masks_d[:])

            # per-window persistent activations
            kT_w = []  # [p(d-pair), kfo(4), TQ] per window, fp16
            v65_w = []  # [p(key), t4(4), HL, 65] per window, fp16 (v | 1)
            for w in range(NTQ):
                kT_w.append(
                    persist.tile([P, 4, TQ], f16, tag=f"kT{w}", name=f"kT{w}")
                )
                v65_w.append(
                    persist.tile(
                        [P, 4, HL, D + 1], f16, tag=f"v65{w}", name=f"v65{w}"
                    )
                )
                nc.vector.memset(v65_w[w][:, :, :, D], 1.0 / 16.0)
            woh_t = persist.tile([P, 4, C], f8)  # [p, do(pair j), n]
            wol_t = persist.tile([P, 4, C], f8)
            for do in range(4):
                nc.gpsimd.dma_start(woh_t[:, do], woh_d[do * P : (do + 1) * P, :])
            for do in range(4):
                nc.gpsimd.dma_start(wol_t[:, do], wol_d[do * P : (do + 1) * P, :])

            # ---------------- unit builders ----------------
            def load_x(w):
                xh_ts, xl_ts = [], []
                for m in range(KOP):
                    t_ = xT_pool.tile([P, 2, TQ], f8, tag=f"xh{m}")
                    nc.sync.dma_start(
                        t_,
                        xh_d[
                            256 * m : 256 * (m + 1), w * TQ : (w + 1) * TQ
                        ].rearrange("(two p) t -> p two t", p=P),
                    )
                    xh_ts.append(t_)
                for m in range(KOP):
                    t_ = xT_pool.tile([P, 2, TQ], f8, tag=f"xl{m}")
                    nc.sync.dma_start(
                        t_,
                        xl_d[
                            256 * m : 256 * (m + 1), w * TQ : (w + 1) * TQ
                        ].rearrange("(two p) t -> p two t", p=P),
                    )
                    xl_ts.append(t_)
                return xh_ts, xl_ts

            def proj_qk_unit(w, fo, qT_next, xh_ts, xl_ts):
                def emit():
                    ps = ps_pj.tile([P, TQ], f32, tag="pj")
                    terms = [(wqkh_t, xh_ts), (wqkl_t, xh_ts), (wqkh_t, xl_ts)]
                    n = 0
                    for wt, xt in terms:
                        for m in range(KOP):
                            nc.tensor.matmul(
                                ps,
                                wt[:, 2 * m : 2 * m + 2, fo * P : (fo + 1) * P],
                                xt[m],
                                start=(n == 0),
                                stop=(n == 3 * KOP - 1),
                                perf_mode=DR,
                            )
                            n += 1
                    dst = qT_next[:, fo] if fo < 4 else kT_w[w][:, fo - 4]
                    nc.vector.tensor_scalar(
                        dst, ps, UNSCALE, b_qk_sb[:, fo : fo + 1], mult, add
                    )

                return emit

            def proj_v_unit(w, t4, xh_ts, xl_ts):
                def emit():
                    ps = ps_pj.tile([P, TQ], f32, tag="pj")
                    terms = [(xh_ts, wvh_t), (xl_ts, wvh_t), (xh_ts, wvl_t)]
                    n = 0
                    for xt, wt in terms:
                        for m in range(KOP):
                            nc.tensor.matmul(
                                ps,
                                xt[m][:, :, t4 * P : (t4 + 1) * P],
                                wt[:, 2 * m : 2 * m + 2, :],
                                start=(n == 0),
                                stop=(n == 3 * KOP - 1),
                                perf_mode=DR,
                            )
                            n += 1
                    nc.vector.tensor_scalar(
                        v65_w[w][:, t4, :, :D],
                        ps.rearrange("p (h d) -> p h d", h=HL),
                        UNSCALE,
                        None,
                        mult,
                    )

                return emit

            def op_unit(tq, ts_, yh_win, yl_win, scalar_copy=False):
                def emit():
                    t0 = tq * TQ + ts_ * P
                    for n in range(2):
                        ps = ps_pj.tile([P, TQ], f32, tag="pj")
                        terms = [(yh_win, woh_t), (yl_win, woh_t), (yh_win, wol_t)]
                        k = 0
                        for m in range(2):
                            for yt, wt in terms:
                                nc.tensor.matmul(
                                    ps,
                                    yt[:, 2 * m : 2 * m + 2, ts_ * P : (ts_ + 1) * P],
                                    wt[:, 2 * m : 2 * m + 2, n * 512 : (n + 1) * 512],
                                    start=(k == 0),
                                    stop=(k == 5),
                                    perf_mode=DR,
                                )
                                k += 1
                        o_sb = o_pool.tile([P, 512], f32, tag="o")
                        if scalar_copy and n == 0:
                            nc.scalar.copy(o_sb, ps)
                        else:
                            nc.vector.tensor_copy(o_sb, ps)
                        q_ = nc.gpsimd if (scalar_copy and n == 1) else nc.sync
                        q_.dma_start(
                            out_d[t0 : t0 + P, n * 512 : (n + 1) * 512], o_sb
                        )

                return emit

            # paced filler drain
            class Pacer:
                def __init__(self, fillers, total_slots, backload=1.0, reserve=0):
                    self.fillers = deque(fillers)
                    self.total = max(1, total_slots)
                    self.n = len(fillers)
                    self.slot = 0
                    self.done = 0
                    self.backload = backload
                    self.reserve = reserve

                def tick(self):
                    self.slot += 1
                    want = min(
                        int(self.n * (self.slot / self.total) ** self.backload),
                        self.n - self.reserve,
                    )
                    while self.done < want and self.fillers:
                        self.fillers.popleft()()
                        self.done += 1

                def drain(self):
                    while self.fillers:
                        self.fillers.popleft()()

            def att_pair(tq, j, qT_cur, yh_win, yl_win, pacer, last=False):
                """Heads 2j (partitions 0:64) and 2j+1 (64:128): S in fp16,
                exp on ACT straight to fp16 pT, PV with moving = pT."""
                nchunks = 4 * (tq + 1)
                hA, hB = 2 * j, 2 * j + 1
                qA = qT_cur[0:D, j, :]
                qB = qT_cur[D:P, j, :]
                psyA = ps_y.tile([D + 1, TQ], f32, tag="psyA", name="psyA")
                psyB = ps_y.tile([D + 1, TQ], f32, tag="psyB", name="psyB")
                from collections import deque as _dq

                pvq = _dq()
                for i in range(nchunks):
                    i4 = i - 4 * tq
                    diag = 0 <= i4 < 4
                    col0 = P * i4 if diag else 0
                    pss = ps_s.tile([P, 2, TQ], f32, tag="ps_s")
                    kslice = slice((i % 4) * P, (i % 4 + 1) * P)
                    nc.tensor.matmul(
                        pss[:, 0, col0:TQ],
                        kT_w[i // 4][0:D, j, kslice],
                        qA[:, col0:TQ],
                        start=True,
                        stop=True,
                    )
                    nc.tensor.matmul(
                        pss[:, 1, col0:TQ],
                        kT_w[i // 4][D:P, j, kslice],
                        qB[:, col0:TQ],
                        start=True,
                        stop=True,
                    )
                    pT = pT_pool.tile([P, 2, TQ], f16, tag="pT")
                    nc.scalar.activation(
                        pT[:, :, col0:TQ], pss[:, :, col0:TQ], Exp
                    )
                    if diag:
                        # only the diagonal 128-col block needs masking
                        nc.vector.tensor_tensor(
                            pT[:, :, col0 : col0 + P],
                            pT[:, :, col0 : col0 + P],
                            mask_sb[:, 384 : 384 + P]
                            .unsqueeze(1)
                            .to_broadcast((P, 2, P)),
                            mult,
                        )
                    pvq.append((pT, i, col0))
                    if len(pvq) > 6:
                        _pv(pvq.popleft(), psyA, psyB, nchunks, hA, hB)
                    pacer.tick()
                while pvq:
                    _pv(pvq.popleft(), psyA, psyB, nchunks, hA, hB)
                # normalize: copy psy to SBUF first (frees the PSUM bank for
                # the next pair), then reciprocal + GpSimd partition
                # broadcast + multiply, A/B interleaved to hide latency
                stage = []
                for psy, pb in ((psyA, 0), (psyB, D)):
                    if last:
                        # final pair: nobody needs the PSUM bank next, so
                        # normalize straight out of PSUM (shorter tail chain)
                        psy_sb = psy
                    else:
                        psy_sb = sm.tile(
                            [D + 1, TQ], f16, tag="psy_sb", name="psy_sb"
                        )
                        nc.vector.tensor_copy(psy_sb, psy)
                    rec_bc = sm.tile([D, TQ], f32, tag="rec_bc")
                    stage.append((psy_sb, rec_bc, pb))
                for psy_sb, rec_bc, pb in stage:
                    with nc.allow_low_precision(reason="softmax denominators"):
                        nc.vector.reciprocal(rec_bc[0:1, :], psy_sb[D : D + 1, :])
                for psy_sb, rec_bc, pb in stage:
                    nc.gpsimd.partition_broadcast(rec_bc, rec_bc[0:1, :], channels=D)
                y16 = sm.tile([P, TQ], f16, tag="y16", name="y16")
                if last:
                    # col-sliced so the tail out-projections start per q-tile
                    for ts_ in range(4):
                        cs = slice(ts_ * P, (ts_ + 1) * P)
                        for psy_sb, rec_bc, pb in stage:
                            nc.vector.tensor_tensor(
                                y16[pb : pb + D, cs],
                                psy_sb[:D, cs],
                                rec_bc[:, cs],
                                mult,
                            )
                            nc.vector.tensor_copy(
                                yh_win[pb : pb + D, j, cs], y16[pb : pb + D, cs]
                            )
                            nc.vector.tensor_tensor(
                                yl_win[pb : pb + D, j, cs],
                                y16[pb : pb + D, cs],
                                yh_win[pb : pb + D, j, cs],
                                subtract,
                            )
                else:
                    for psy_sb, rec_bc, pb in stage:
                        nc.vector.tensor_tensor(
                            y16[pb : pb + D, :], psy_sb[:D, :], rec_bc, mult
                        )
                        # split into fp8 hi+lo (16*y fits e4m3 range);
                        # alternate engines so DVE and Pool share the work
                        eng = nc.vector if pb == 0 else nc.gpsimd
                        eng.tensor_copy(
                            yh_win[pb : pb + D, j, :], y16[pb : pb + D, :]
                        )
                        eng.tensor_tensor(
                            yl_win[pb : pb + D, j, :],
                            y16[pb : pb + D, :],
                            yh_win[pb : pb + D, j, :],
                            subtract,
                        )

            def _pv(prev, psyA, psyB, nchunks, hA, hB):
                pT, i, col0 = prev
                nc.tensor.matmul(
                    psyA[:, col0:TQ],
                    v65_w[i // 4][:, i % 4, hA],
                    pT[:, 0, col0:TQ],
                    start=(i == 0),
                    stop=(i == nchunks - 1),
                )
                nc.tensor.matmul(
                    psyB[:, col0:TQ],
                    v65_w[i // 4][:, i % 4, hB],
                    pT[:, 1, col0:TQ],
                    start=(i == 0),
                    stop=(i == nchunks - 1),
                )

            # ---------------- emission ----------------
            # window-0 projection: ko-pair-outer so PE starts on first chunks
            qT_cur = qT_pool.tile([P, 4, TQ], tag="qT", dtype=f16)
            with tc.tile_pool(name="pj0", bufs=1, space="PSUM") as pj0:
                ps_fo = [
                    pj0.tile([P, TQ], f32, tag=f"pj0_{fo}", name=f"pj0_{fo}")
                    for fo in range(KO)
                ]
                sweeps = [(wqkh_t, xh0), (wqkl_t, xh0), (wqkh_t, xl0)]
                fo_last = [4, 0, 1, 2, 3, 5, 6, 7]  # kT chunk 0 + qT first
                for s, (wt, xt) in enumerate(sweeps):
                    for m in range(KOP):
                        last = s == 2 and m == KOP - 1
                        for fo in fo_last if last else range(KO):
                            nc.tensor.matmul(
                                ps_fo[fo],
                                wt[:, 2 * m : 2 * m + 2, fo * P : (fo + 1) * P],
                                xt[m],
                                start=(s == 0 and m == 0),
                                stop=last,
                                perf_mode=DR,
                            )
                            if last:
                                dst = (
                                    qT_cur[:, fo] if fo < 4 else kT_w[0][:, fo - 4]
                                )
                                nc.vector.tensor_scalar(
                                    dst,
                                    ps_fo[fo],
                                    UNSCALE,
                                    b_qk_sb[:, fo : fo + 1],
                                    mult,
                                    add,
                                )
                for t4 in range(4):
                    psv = pj0.tile([P, TQ], f32, tag=f"pj0_{t4}", name=f"pj0v_{t4}")
                    vsweeps = [(xh0, wvh_t), (xl0, wvh_t), (xh0, wvl_t)]
                    n = 0
                    for xt, wt in vsweeps:
                        for m in range(KOP):
                            nc.tensor.matmul(
                                psv,
                                xt[m][:, :, t4 * P : (t4 + 1) * P],
                                wt[:, 2 * m : 2 * m + 2, :],
                                start=(n == 0),
                                stop=(n == 3 * KOP - 1),
                                perf_mode=DR,
                            )
                            n += 1
                    nc.vector.tensor_scalar(
                        v65_w[0][:, t4, :, :D],
                        psv.rearrange("p (h d) -> p h d", h=HL),
                        UNSCALE,
                        None,
                        mult,
                    )
            ps_pj = ctx.enter_context(tc.tile_pool(name="ps_pj", bufs=2, space="PSUM"))
            ps_s = ctx.enter_context(tc.tile_pool(name="ps_s", bufs=2, space="PSUM"))
            ps_y = ctx.enter_context(tc.tile_pool(name="ps_y", bufs=1, space="PSUM"))

            yT_all = []
            for tq in range(NTQ):
                fillers = []
                qT_next = None
                if tq + 1 < NTQ:
                    xh_ts, xl_ts = load_x(tq + 1)
                    qT_next = qT_pool.tile([P, 4, TQ], tag="qT", dtype=f16)
                    # consumption order at window start: kT chunk, qT pair, v
                    for fo, t4 in ((4, 0), (0, 1), (5, 2), (1, 3)):
                        fillers.append(
                            proj_qk_unit(tq + 1, fo, qT_next, xh_ts, xl_ts)
                        )
                        fillers.append(proj_v_unit(tq + 1, t4, xh_ts, xl_ts))
                    for fo in (6, 2, 7, 3):
                        fillers.append(
                            proj_qk_unit(tq + 1, fo, qT_next, xh_ts, xl_ts)
                        )
                else:
                    # window 3 is ACT(exp)-bound: all deferred out-projections
                    # (windows 0-2) become its PE filler mass
                    for w in range(NTQ - 1):
                        for ts_ in range(4):
                            fillers.append(op_unit(w, ts_, *yT_all[w]))
                yh_win = yT_pool.tile([P, 4, TQ], f8, tag="yh", name="yh_win")
                yl_win = yT_pool.tile([P, 4, TQ], f8, tag="yl", name="yl_win")
                yT_all.append((yh_win, yl_win))
                pacer = Pacer(
                    fillers,
                    total_slots=4 * 4 * (tq + 1),
                    backload=1.0,
                    reserve=0,
                )
                if tq == 0 and fillers:
                    for _ in range(2):
                        pacer.fillers.popleft()()
                        pacer.done += 1
                for j in range(HL // 2):
                    att_pair(
                        tq,
                        j,
                        qT_cur,
                        yh_win,
                        yl_win,
                        pacer,
                        last=(tq == NTQ - 1 and j == HL // 2 - 1),
                    )
                pacer.drain()
                qT_cur = qT_next
            for ts_ in range(4):
                op_unit(NTQ - 1, ts_, *yT_all[NTQ - 1], scalar_copy=True)()

    nc.compile()
    return nc


def _get_nc():
    if "nc" not in _CACHE:
        _CACHE["nc"] = _build()
    return _CACHE["nc"]


def _split8(a, s):
    import ml_dtypes

    E4 = ml_dtypes.float8_e4m3
    a = np.asarray(a, np.float32) * np.float32(s)
    h = a.astype(E4)
    l = (a - h.astype(np.float32)).astype(E4)
    return np.ascontiguousarray(h), np.ascontiguousarray(l)


def kernel(x, W_in, b_in, W_out, b_out):
    from concourse.bass_utils import run_bass_kernel_spmd

    x = np.asarray(x, dtype=np.float32)
    W_in = np.asarray(W_in, dtype=np.float32)
    b_in = np.asarray(b_in, dtype=np.float32)
    W_out = np.asarray(W_out, dtype=np.float32)
    b_out = np.asarray(b_out, dtype=np.float32)

    scale = 1.0 / np.sqrt(D)

    # causal mask master: M[p, u] = 1 if u >= p + 384
    u = np.arange(896)[None, :]
    p = np.arange(P)[:, None]
    mask = (u >= p + 384).astype(np.float16)

    in_maps = []
    for c in range(8):
        b, g = c // 2, c % 2
        qc = slice(g * HL * D, (g + 1) * HL * D)
        kc = slice(C + g * HL * D, C + (g + 1) * HL * D)
        vc = slice(2 * C + g * HL * D, 2 * C + (g + 1) * HL * D)
        w_qk = np.concatenate([W_in[:, qc] * scale, W_in[:, kc]], axis=1)
        b_qk = np.concatenate([b_in[qc] * scale, b_in[kc]]).astype(np.float32)
        wqkh, wqkl = _split8(w_qk, SW)
        wvh, wvl = _split8(W_in[:, vc], SW)
        woh, wol = _split8(W_out[g * HL * D : (g + 1) * HL * D, :], SW)
        xh, xl = _split8(x[b].T, SX)
        in_maps.append(
            {
                "xh": xh,
                "xl": xl,
                "wqkh": wqkh,
                "wqkl": wqkl,
                "wvh": wvh,
                "wvl": wvl,
                "woh": woh,
                "wol": wol,
                "b_qk": b_qk,
                "masks": mask,
            }
        )

    nc = _get_nc()
    res = run_bass_kernel_spmd(nc, in_maps, list(range(8)))

    # host epilogue: add halves, biases, and the v-bias term
    # (y = softmax(..)@(v + b_v) = y0 + b_v, so b_v @ W_out adds to out)
    bv_term = np.zeros(C, np.float64)
    for g in range(2):
        vc = slice(2 * C + g * HL * D, 2 * C + (g + 1) * HL * D)
        gsl = slice(g * HL * D, (g + 1) * HL * D)
        bv_term += b_in[vc].astype(np.float64) @ W_out[gsl].astype(np.float64)
    epilogue = (bv_term + b_out).astype(np.float32)

    out = np.empty((B, T, C), np.float32)
    unscale_out = np.float32(1.0 / (16.0 * SW))
    for b in range(B):
        out[b] = (
            res.results[2 * b]["out"] + res.results[2 * b + 1]["out"]
        ) * unscale_out + epilogue
    return out


if __name__ == "__main__":
    rng = np.random.default_rng(0)
    x = rng.standard_normal((B, T, C), dtype=np.float32)
    W_in = rng.standard_normal((C, 3 * C), dtype=np.float32) / np.sqrt(C)
    b_in = np.zeros(3 * C, np.float32)
    W_out = rng.standard_normal((C, C), dtype=np.float32) / np.sqrt(C)
    b_out = np.zeros(C, np.float32)
    y = kernel(x=x, W_in=W_in, b_in=b_in, W_out=W_out, b_out=b_out)
    print("ok", y.shape, y.dtype)


# revision 41
# speedup vs baseline: 1.0020x; 1.0006x over previous
"""Causal self-attention on 8 TRN2 NeuronCores.

Problem (hardcoded): B=4, T=2048, C=1024, H=16 heads, D=64.
  qkv = x @ W_in + b_in ; causal softmax attention ; out = y @ W_out + b_out

Sharding: core c handles batch b = c//2 and head-group g = c%2 (8 heads).
Each core computes its partial out-projection (sum over its heads' columns);
the host adds the two partials per batch plus b_out (+ the v-bias term,
which commutes through the softmax average). No device collectives.

Device design (fp8 projections, fp16 attention core):
  - QKV projection runs as fp8e4 DoubleRow matmuls: x and W_in are host-split
    into hi+lo e4m3 parts (pre-scaled by 2^5 / 2^7 so values sit mid-range,
    away from fp8 subnormals), and 3 of the 4 cross terms are accumulated
    (hi*hi + lo*hi + hi*lo). DoubleRow contracts two 128-chunks per
    instruction at 0.5 cycles/row -> 1.33x the fp16 rate at ~0.1% error.
    The 2^-12 unscale folds into the bias-add (tensor_scalar mult+add).
  - Attention core is fp16: q/k (1/sqrt(D) folded into W_q), P = exp(S) from
    ACT directly in fp16 (max P ~500 fits fp16; no max-subtraction), v
    carries a ones-column so the PV matmul emits softmax denominators free.
  - Scores computed transposed: S^T[k, q] = k . q, so exp(S^T) = P^T feeds
    the PV matmul directly -- no on-chip transposes anywhere.
  - Causal masking multiplies only the 128-wide diagonal block of each
    diagonal chunk (columns left of it are skipped via col0, columns right
    of it are fully valid).
  - Causal pipeline: projection of query-window w+1 and the out-projection
    of window w-1 are emitted as PE "filler" units interleaved into the
    ACT(exp)-bound attention stream of window w, keeping PE dense.
"""

import sys

for _p in ("/opt/trn_rl_repo", "/root/.axon_site/_ro/trn_rl_repo"):
    if _p not in sys.path:
        sys.path.append(_p)

import numpy as np

B, T, C = 4, 2048, 1024
H = 16  # total heads
HL = 8  # heads per core
D = 64  # head dim
P = 128
KO = C // P  # 8 contraction chunks
KOP = KO // 2  # 4 DoubleRow chunk-pairs
TQ = 512  # query-window width
NTQ = T // TQ  # 4 windows
FQK = 2 * HL * D  # 1024 (q block then k block)
FV = HL * D  # 512

# fp8 pre-scales (powers of two; exact)
SX = 32.0  # x scale
SW = 128.0  # weight scale
UNSCALE = 1.0 / (SX * SW)  # folded into the bias-add after projection

_CACHE = {}


def _build():
    import concourse.mybir as mybir
    import concourse.tile as tile
    from concourse import bacc

    f32 = mybir.dt.float32
    f16 = mybir.dt.float16
    f8 = mybir.dt.float8e4
    DR = mybir.MatmulPerfMode.DoubleRow
    Exp = mybir.ActivationFunctionType.Exp
    mult = mybir.AluOpType.mult
    add = mybir.AluOpType.add
    subtract = mybir.AluOpType.subtract

    nc = bacc.Bacc("TRN2", target_bir_lowering=False, debug=False, num_devices=8)

    xh_d = nc.dram_tensor("xh", [C, T], f8, kind="ExternalInput")
    xl_d = nc.dram_tensor("xl", [C, T], f8, kind="ExternalInput")
    wqkh_d = nc.dram_tensor("wqkh", [C, FQK], f8, kind="ExternalInput")
    wqkl_d = nc.dram_tensor("wqkl", [C, FQK], f8, kind="ExternalInput")
    wvh_d = nc.dram_tensor("wvh", [C, FV], f8, kind="ExternalInput")
    wvl_d = nc.dram_tensor("wvl", [C, FV], f8, kind="ExternalInput")
    woh_d = nc.dram_tensor("woh", [HL * D, C], f8, kind="ExternalInput")
    wol_d = nc.dram_tensor("wol", [HL * D, C], f8, kind="ExternalInput")
    b_qk_d = nc.dram_tensor("b_qk", [FQK], f32, kind="ExternalInput")
    masks_d = nc.dram_tensor("masks", [P, 896], f16, kind="ExternalInput")
    out_d = nc.dram_tensor("out", [T, C], f32, kind="ExternalOutput")

    with tile.TileContext(nc) as tc:
        import contextlib
        from collections import deque

        ctx = contextlib.ExitStack()
        with ctx:
            persist = ctx.enter_context(tc.tile_pool(name="persist", bufs=1))
            qT_pool = ctx.enter_context(tc.tile_pool(name="qT", bufs=2))
            xT_pool = ctx.enter_context(tc.tile_pool(name="xT", bufs=2))
            pT_pool = ctx.enter_context(tc.tile_pool(name="pT", bufs=8))
            sm = ctx.enter_context(tc.tile_pool(name="sm", bufs=6))
            yT_pool = ctx.enter_context(tc.tile_pool(name="yT", bufs=4))
            o_pool = ctx.enter_context(tc.tile_pool(name="o", bufs=8))

            # ---- weights + first x window, in first-use order ----
            wqkh_t = persist.tile([P, KO, FQK], f8)
            wqkl_t = persist.tile([P, KO, FQK], f8)
            xh0, xl0 = [], []
            for m in range(KOP):
                nc.sync.dma_start(
                    wqkh_t[:, 2 * m : 2 * m + 2],
                    wqkh_d[256 * m : 256 * (m + 1)].rearrange(
                        "(two p) f -> p two f", p=P
                    ),
                )
                t_ = xT_pool.tile([P, 2, TQ], f8, tag=f"xh{m}", name=f"xh0_{m}")
                nc.gpsimd.dma_start(
                    t_,
                    xh_d[256 * m : 256 * (m + 1), 0:TQ].rearrange(
                        "(two p) t -> p two t", p=P
                    ),
                )
                xh0.append(t_)
            for m in range(KOP):
                nc.sync.dma_start(
                    wqkl_t[:, 2 * m : 2 * m + 2],
                    wqkl_d[256 * m : 256 * (m + 1)].rearrange(
                        "(two p) f -> p two f", p=P
                    ),
                )
                t_ = xT_pool.tile([P, 2, TQ], f8, tag=f"xl{m}", name=f"xl0_{m}")
                # split the lo-x loads across both queues so the last chunk
                # lands early (the w0 projection is paced by it)
                q_ = nc.gpsimd if m < 2 else nc.sync
                q_.dma_start(
                    t_,
                    xl_d[256 * m : 256 * (m + 1), 0:TQ].rearrange(
                        "(two p) t -> p two t", p=P
                    ),
                )
                xl0.append(t_)
            b_qk_sb = persist.tile([P, KO], f32)
            nc.sync.dma_start(b_qk_sb, b_qk_d.rearrange("(fo p) -> p fo", p=P))
            # preload the Exp activation table off the critical path
            warm = persist.tile([1, 1], f32)
            nc.scalar.activation(warm, b_qk_sb[0:1, 0:1], Exp)
            wvh_t = persist.tile([P, KO, FV], f8)
            wvl_t = persist.tile([P, KO, FV], f8)
            for m in range(KOP):
                nc.gpsimd.dma_start(
                    wvh_t[:, 2 * m : 2 * m + 2],
                    wvh_d[256 * m : 256 * (m + 1)].rearrange(
                        "(two p) f -> p two f", p=P
                    ),
                )
            for m in range(KOP):
                nc.gpsimd.dma_start(
                    wvl_t[:, 2 * m : 2 * m + 2],
                    wvl_d[256 * m : 256 * (m + 1)].rearrange(
                        "(two p) f -> p two f", p=P
                    ),
                )
            mask_sb = persist.tile([P, 896], f16)
            nc.sync.dma_start(mask_sb, masks_d[:])

            # per-window persistent activations
            kT_w = []  # [p(d-pair), kfo(4), TQ] per window, fp16
            v65_w = []  # [p(key), t4(4), HL, 65] per window, fp16 (v | 1)
            for w in range(NTQ):
                kT_w.append(
                    persist.tile([P, 4, TQ], f16, tag=f"kT{w}", name=f"kT{w}")
                )
                v65_w.append(
                    persist.tile(
                        [P, 4, HL, D + 1], f16, tag=f"v65{w}", name=f"v65{w}"
                    )
                )
                nc.vector.memset(v65_w[w][:, :, :, D], 1.0 / 16.0)
            woh_t = persist.tile([P, 4, C], f8)  # [p, do(pair j), n]
            wol_t = persist.tile([P, 4, C], f8)
            for do in range(4):
                nc.gpsimd.dma_start(woh_t[:, do], woh_d[do * P : (do + 1) * P, :])
            for do in range(4):
                nc.gpsimd.dma_start(wol_t[:, do], wol_d[do * P : (do + 1) * P, :])

            # ---------------- unit builders ----------------
            def load_x(w):
                xh_ts, xl_ts = [], []
                for m in range(KOP):
                    t_ = xT_pool.tile([P, 2, TQ], f8, tag=f"xh{m}")
                    nc.sync.dma_start(
                        t_,
                        xh_d[
                            256 * m : 256 * (m + 1), w * TQ : (w + 1) * TQ
                        ].rearrange("(two p) t -> p two t", p=P),
                    )
                    xh_ts.append(t_)
                for m in range(KOP):
                    t_ = xT_pool.tile([P, 2, TQ], f8, tag=f"xl{m}")
                    nc.sync.dma_start(
                        t_,
                        xl_d[
                            256 * m : 256 * (m + 1), w * TQ : (w + 1) * TQ
                        ].rearrange("(two p) t -> p two t", p=P),
                    )
                    xl_ts.append(t_)
                return xh_ts, xl_ts

            def proj_qk_unit(w, fo, qT_next, xh_ts, xl_ts):
                def emit():
                    ps = ps_pj.tile([P, TQ], f32, tag="pj")
                    terms = [(wqkh_t, xh_ts), (wqkl_t, xh_ts), (wqkh_t, xl_ts)]
                    n = 0
                    for wt, xt in terms:
                        for m in range(KOP):
                            nc.tensor.matmul(
                                ps,
                                wt[:, 2 * m : 2 * m + 2, fo * P : (fo + 1) * P],
                                xt[m],
                                start=(n == 0),
                                stop=(n == 3 * KOP - 1),
                                perf_mode=DR,
                            )
                            n += 1
                    dst = qT_next[:, fo] if fo < 4 else kT_w[w][:, fo - 4]
                    nc.vector.tensor_scalar(
                        dst, ps, UNSCALE, b_qk_sb[:, fo : fo + 1], mult, add
                    )

                return emit

            def proj_v_unit(w, t4, xh_ts, xl_ts):
                def emit():
                    ps = ps_pj.tile([P, TQ], f32, tag="pj")
                    terms = [(xh_ts, wvh_t), (xl_ts, wvh_t), (xh_ts, wvl_t)]
                    n = 0
                    for xt, wt in terms:
                        for m in range(KOP):
                            nc.tensor.matmul(
                                ps,
                                xt[m][:, :, t4 * P : (t4 + 1) * P],
                                wt[:, 2 * m : 2 * m + 2, :],
                                start=(n == 0),
                                stop=(n == 3 * KOP - 1),
                                perf_mode=DR,
                            )
                            n += 1
                    nc.vector.tensor_scalar(
                        v65_w[w][:, t4, :, :D],
                        ps.rearrange("p (h d) -> p h d", h=HL),
                        UNSCALE,
                        None,
                        mult,
                    )

                return emit

            def op_unit(tq, ts_, yh_win, yl_win, scalar_copy=False):
                def emit():
                    t0 = tq * TQ + ts_ * P
                    for n in range(2):
                        ps = ps_pj.tile([P, TQ], f32, tag="pj")
                        terms = [(yh_win, woh_t), (yl_win, woh_t), (yh_win, wol_t)]
                        k = 0
                        for m in range(2):
                            for yt, wt in terms:
                                nc.tensor.matmul(
                                    ps,
                                    yt[:, 2 * m : 2 * m + 2, ts_ * P : (ts_ + 1) * P],
                                    wt[:, 2 * m : 2 * m + 2, n * 512 : (n + 1) * 512],
                                    start=(k == 0),
                                    stop=(k == 5),
                                    perf_mode=DR,
                                )
                                k += 1
                        o_sb = o_pool.tile([P, 512], f32, tag="o")
                        if scalar_copy and n == 0:
                            nc.scalar.copy(o_sb, ps)
                        else:
                            nc.vector.tensor_copy(o_sb, ps)
                        q_ = nc.gpsimd if (scalar_copy and n == 1) else nc.sync
                        q_.dma_start(
                            out_d[t0 : t0 + P, n * 512 : (n + 1) * 512], o_sb
                        )

                return emit

            # paced filler drain
            class Pacer:
                def __init__(self, fillers, total_slots, backload=1.0, reserve=0):
                    self.fillers = deque(fillers)
                    self.total = max(1, total_slots)
                    self.n = len(fillers)
                    self.slot = 0
                    self.done = 0
                    self.backload = backload
                    self.reserve = reserve

                def tick(self):
                    self.slot += 1
                    want = min(
                        int(self.n * (self.slot / self.total) ** self.backload),
                        self.n - self.reserve,
                    )
                    while self.done < want and self.fillers:
                        self.fillers.popleft()()
                        self.done += 1

                def drain(self):
                    while self.fillers:
                        self.fillers.popleft()()

            def att_pair(tq, j, qT_cur, yh_win, yl_win, pacer, last=False):
                """Heads 2j (partitions 0:64) and 2j+1 (64:128): S in fp16,
                exp on ACT straight to fp16 pT, PV with moving = pT."""
                nchunks = 4 * (tq + 1)
                hA, hB = 2 * j, 2 * j + 1
                qA = qT_cur[0:D, j, :]
                qB = qT_cur[D:P, j, :]
                psyA = ps_y.tile([D + 1, TQ], f32, tag="psyA", name="psyA")
                psyB = ps_y.tile([D + 1, TQ], f32, tag="psyB", name="psyB")
                from collections import deque as _dq

                pvq = _dq()
                for i in range(nchunks):
                    i4 = i - 4 * tq
                    diag = 0 <= i4 < 4
                    col0 = P * i4 if diag else 0
                    pss = ps_s.tile([P, 2, TQ], f32, tag="ps_s")
                    kslice = slice((i % 4) * P, (i % 4 + 1) * P)
                    nc.tensor.matmul(
                        pss[:, 0, col0:TQ],
                        kT_w[i // 4][0:D, j, kslice],
                        qA[:, col0:TQ],
                        start=True,
                        stop=True,
                    )
                    nc.tensor.matmul(
                        pss[:, 1, col0:TQ],
                        kT_w[i // 4][D:P, j, kslice],
                        qB[:, col0:TQ],
                        start=True,
                        stop=True,
                    )
                    pT = pT_pool.tile([P, 2, TQ], f16, tag="pT")
                    nc.scalar.activation(
                        pT[:, :, col0:TQ], pss[:, :, col0:TQ], Exp
                    )
                    if diag:
                        # only the diagonal 128-col block needs masking
                        nc.vector.tensor_tensor(
                            pT[:, :, col0 : col0 + P],
                            pT[:, :, col0 : col0 + P],
                            mask_sb[:, 384 : 384 + P]
                            .unsqueeze(1)
                            .to_broadcast((P, 2, P)),
                            mult,
                        )
                    pvq.append((pT, i, col0))
                    if len(pvq) > 6:
                        _pv(pvq.popleft(), psyA, psyB, nchunks, hA, hB)
                    pacer.tick()
                while pvq:
                    _pv(pvq.popleft(), psyA, psyB, nchunks, hA, hB)
                # normalize: copy psy to SBUF first (frees the PSUM bank for
                # the next pair), then reciprocal + GpSimd partition
                # broadcast + multiply, A/B interleaved to hide latency
                stage = []
                for psy, pb in ((psyA, 0), (psyB, D)):
                    if last:
                        # final pair: nobody needs the PSUM bank next, so
                        # normalize straight out of PSUM (shorter tail chain)
                        psy_sb = psy
                    else:
                        psy_sb = sm.tile(
                            [D + 1, TQ], f16, tag="psy_sb", name="psy_sb"
                        )
                        nc.vector.tensor_copy(psy_sb, psy)
                    rec_bc = sm.tile([D, TQ], f32, tag="rec_bc")
                    stage.append((psy_sb, rec_bc, pb))
                for psy_sb, rec_bc, pb in stage:
                    with nc.allow_low_precision(reason="softmax denominators"):
                        nc.vector.reciprocal(rec_bc[0:1, :], psy_sb[D : D + 1, :])
                for psy_sb, rec_bc, pb in stage:
                    nc.gpsimd.partition_broadcast(rec_bc, rec_bc[0:1, :], channels=D)
                y16 = sm.tile([P, TQ], f16, tag="y16", name="y16")
                if last:
                    # col-sliced so the tail out-projections start per q-tile
                    for ts_ in range(4):
                        cs = slice(ts_ * P, (ts_ + 1) * P)
                        for psy_sb, rec_bc, pb in stage:
                            nc.vector.tensor_tensor(
                                y16[pb : pb + D, cs],
                                psy_sb[:D, cs],
                                rec_bc[:, cs],
                                mult,
                            )
                            nc.vector.tensor_copy(
                                yh_win[pb : pb + D, j, cs], y16[pb : pb + D, cs]
                            )
                            nc.vector.tensor_tensor(
                                yl_win[pb : pb + D, j, cs],
                                y16[pb : pb + D, cs],
                                yh_win[pb : pb + D, j, cs],
                                subtract,
                            )
                else:
                    for psy_sb, rec_bc, pb in stage:
                        nc.vector.tensor_tensor(
                            y16[pb : pb + D, :], psy_sb[:D, :], rec_bc, mult
                        )
                        # split into fp8 hi+lo (16*y fits e4m3 range);
                        # alternate engines so DVE and Pool share the work
                        eng = nc.vector if pb == 0 else nc.gpsimd
                        eng.tensor_copy(
                            yh_win[pb : pb + D, j, :], y16[pb : pb + D, :]
                        )
                        eng.tensor_tensor(
                            yl_win[pb : pb + D, j, :],
                            y16[pb : pb + D, :],
                            yh_win[pb : pb + D, j, :],
                            subtract,
                        )

            def _pv(prev, psyA, psyB, nchunks, hA, hB):
                pT, i, col0 = prev
                nc.tensor.matmul(
                    psyA[:, col0:TQ],
                    v65_w[i // 4][:, i % 4, hA],
                    pT[:, 0, col0:TQ],
                    start=(i == 0),
                    stop=(i == nchunks - 1),
                )
                nc.tensor.matmul(
                    psyB[:, col0:TQ],
                    v65_w[i // 4][:, i % 4, hB],
                    pT[:, 1, col0:TQ],
                    start=(i == 0),
                    stop=(i == nchunks - 1),
                )

            # ---------------- emission ----------------
            # window-0 projection: ko-pair-outer so PE starts on first chunks
            qT_cur = qT_pool.tile([P, 4, TQ], tag="qT", dtype=f16)
            with tc.tile_pool(name="pj0", bufs=1, space="PSUM") as pj0:
                ps_fo = [
                    pj0.tile([P, TQ], f32, tag=f"pj0_{fo}", name=f"pj0_{fo}")
                    for fo in range(KO)
                ]
                sweeps = [(wqkh_t, xh0), (wqkl_t, xh0), (wqkh_t, xl0)]
                fo_last = [4, 0, 1, 2, 3, 5, 6, 7]  # kT chunk 0 + qT first
                for s, (wt, xt) in enumerate(sweeps):
                    for m in range(KOP):
                        last = s == 2 and m == KOP - 1
                        for fo in fo_last if last else range(KO):
                            nc.tensor.matmul(
                                ps_fo[fo],
                                wt[:, 2 * m : 2 * m + 2, fo * P : (fo + 1) * P],
                                xt[m],
                                start=(s == 0 and m == 0),
                                stop=last,
                                perf_mode=DR,
                            )
                            if last:
                                dst = (
                                    qT_cur[:, fo] if fo < 4 else kT_w[0][:, fo - 4]
                                )
                                nc.vector.tensor_scalar(
                                    dst,
                                    ps_fo[fo],
                                    UNSCALE,
                                    b_qk_sb[:, fo : fo + 1],
                                    mult,
                                    add,
                                )
                for t4 in range(4):
                    psv = pj0.tile([P, TQ], f32, tag=f"pj0_{t4}", name=f"pj0v_{t4}")
                    vsweeps = [(xh0, wvh_t), (xl0, wvh_t), (xh0, wvl_t)]
                    n = 0
                    for xt, wt in vsweeps:
                        for m in range(KOP):
                            nc.tensor.matmul(
                                psv,
                                xt[m][:, :, t4 * P : (t4 + 1) * P],
                                wt[:, 2 * m : 2 * m + 2, :],
                                start=(n == 0),
                                stop=(n == 3 * KOP - 1),
                                perf_mode=DR,
                            )
                            n += 1
                    nc.vector.tensor_scalar(
                        v65_w[0][:, t4, :, :D],
                        psv.rearrange("p (h d) -> p h d", h=HL),
                        UNSCALE,
                        None,
                        mult,
                    )
            ps_pj = ctx.enter_context(tc.tile_pool(name="ps_pj", bufs=2, space="PSUM"))
            ps_s = ctx.enter_context(tc.tile_pool(name="ps_s", bufs=2, space="PSUM"))
            ps_y = ctx.enter_context(tc.tile_pool(name="ps_y", bufs=1, space="PSUM"))

            yT_all = []
            for tq in range(NTQ):
                fillers = []
                qT_next = None
                if tq + 1 < NTQ:
                    xh_ts, xl_ts = load_x(tq + 1)
                    qT_next = qT_pool.tile([P, 4, TQ], tag="qT", dtype=f16)
                    # consumption order at window start: kT chunk, qT pair, v
                    for fo, t4 in ((4, 0), (0, 1), (5, 2), (1, 3)):
                        fillers.append(
                            proj_qk_unit(tq + 1, fo, qT_next, xh_ts, xl_ts)
                        )
                        fillers.append(proj_v_unit(tq + 1, t4, xh_ts, xl_ts))
                    for fo in (6, 2, 7, 3):
                        fillers.append(
                            proj_qk_unit(tq + 1, fo, qT_next, xh_ts, xl_ts)
                        )
                else:
                    # window 3 is ACT(exp)-bound: all deferred out-projections
                    # (windows 0-2) become its PE filler mass
                    for w in range(NTQ - 1):
                        for ts_ in range(4):
                            fillers.append(op_unit(w, ts_, *yT_all[w]))
                yh_win = yT_pool.tile([P, 4, TQ], f8, tag="yh", name="yh_win")
                yl_win = yT_pool.tile([P, 4, TQ], f8, tag="yl", name="yl_win")
                yT_all.append((yh_win, yl_win))
                pacer = Pacer(
                    fillers,
                    total_slots=4 * 4 * (tq + 1),
                    backload=1.15 if tq in (1, 2) else 1.0,
                    reserve=0,
                )
                if tq == 0 and fillers:
                    for _ in range(2):
                        pacer.fillers.popleft()()
                        pacer.done += 1
                for j in range(HL // 2):
                    att_pair(
                        tq,
                        j,
                        qT_cur,
                        yh_win,
                        yl_win,
                        pacer,
                        last=(tq == NTQ - 1 and j == HL // 2 - 1),
                    )
                pacer.drain()
                qT_cur = qT_next
            for ts_ in range(4):
                op_unit(NTQ - 1, ts_, *yT_all[NTQ - 1], scalar_copy=True)()

    nc.compile()
    return nc


def _get_nc():
    if "nc" not in _CACHE:
        _CACHE["nc"] = _build()
    return _CACHE["nc"]


def _split8(a, s):
    import ml_dtypes

    E4 = ml_dtypes.float8_e4m3
    a = np.asarray(a, np.float32) * np.float32(s)
    h = a.astype(E4)
    l = (a - h.astype(np.float32)).astype(E4)
    return np.ascontiguousarray(h), np.ascontiguousarray(l)


def kernel(x, W_in, b_in, W_out, b_out):
    from concourse.bass_utils import run_bass_kernel_spmd

    x = np.asarray(x, dtype=np.float32)
    W_in = np.asarray(W_in, dtype=np.float32)
    b_in = np.asarray(b_in, dtype=np.float32)
    W_out = np.asarray(W_out, dtype=np.float32)
    b_out = np.asarray(b_out, dtype=np.float32)

    scale = 1.0 / np.sqrt(D)

    # causal mask master: M[p, u] = 1 if u >= p + 384
    u = np.arange(896)[None, :]
    p = np.arange(P)[:, None]
    mask = (u >= p + 384).astype(np.float16)

    in_maps = []
    for c in range(8):
        b, g = c // 2, c % 2
        qc = slice(g * HL * D, (g + 1) * HL * D)
        kc = slice(C + g * HL * D, C + (g + 1) * HL * D)
        vc = slice(2 * C + g * HL * D, 2 * C + (g + 1) * HL * D)
        w_qk = np.concatenate([W_in[:, qc] * scale, W_in[:, kc]], axis=1)
        b_qk = np.concatenate([b_in[qc] * scale, b_in[kc]]).astype(np.float32)
        wqkh, wqkl = _split8(w_qk, SW)
        wvh, wvl = _split8(W_in[:, vc], SW)
        woh, wol = _split8(W_out[g * HL * D : (g + 1) * HL * D, :], SW)
        xh, xl = _split8(x[b].T, SX)
        in_maps.append(
            {
                "xh": xh,
                "xl": xl,
                "wqkh": wqkh,
                "wqkl": wqkl,
                "wvh": wvh,
                "wvl": wvl,
                "woh": woh,
                "wol": wol,
                "b_qk": b_qk,
                "masks": mask,
            }
        )

    nc = _get_nc()
    res = run_bass_kernel_spmd(nc, in_maps, list(range(8)))

    # host epilogue: add halves, biases, and the v-bias term
    # (y = softmax(..)@(v + b_v) = y0 + b_v, so b_v @ W_out adds to out)
    bv_term = np.zeros(C, np.float64)
    for g in range(2):
        vc = slice(2 * C + g * HL * D, 2 * C + (g + 1) * HL * D)
        gsl = slice(g * HL * D, (g + 1) * HL * D)
        bv_term += b_in[vc].astype(np.float64) @ W_out[gsl].astype(np.float64)
    epilogue = (bv_term + b_out).astype(np.float32)

    out = np.empty((B, T, C), np.float32)
    unscale_out = np.float32(1.0 / (16.0 * SW))
    for b in range(B):
        out[b] = (
            res.results[2 * b]["out"] + res.results[2 * b + 1]["out"]
        ) * unscale_out + epilogue
    return out


if __name__ == "__main__":
    rng = np.random.default_rng(0)
    x = rng.standard_normal((B, T, C), dtype=np.float32)
    W_in = rng.standard_normal((C, 3 * C), dtype=np.float32) / np.sqrt(C)
    b_in = np.zeros(3 * C, np.float32)
    W_out = rng.standard_normal((C, C), dtype=np.float32) / np.sqrt(C)
    b_out = np.zeros(C, np.float32)
    y = kernel(x=x, W_in=W_in, b_in=b_in, W_out=W_out, b_out=b_out)
    print("ok", y.shape, y.dtype)
